# revision 40
# baseline (speedup 1.0000x reference)
"""Trainium2 Bass kernel for nn_CrossAttention (LoRA cross-attention).

Sharding: data-parallel over batch — 16 batches across 8 cores, 2 per core.

Per-core strategy (all feature-major, no data transposes of activations):
  - z arrives [C=64, HW=4096] per batch (native layout) = q-proj rhs.
  - q is projected f32r then cast (+bias) to fp8e4 on DVE; k is projected
    bf16 and cast (+bias) to fp8e4 on ACT. Scores then run as fp8
    DoubleRow matmuls (0.5 cyc/col) against a block-diagonal packed K
    stationary: all 8 heads x 77 keys bin-packed into 5 groups of <=128
    PSUM partitions, so exp() runs at full 128-lane ACT utilization.
  - The augmented block-diagonal V stationary (bf16) carries 64 replicated
    indicator columns, so the AV PSUM tile holds features in rows 0:64 and
    the per-head softmax sums replicated x8 in rows 64:128 — normalization
    is then a single DVE divide per chunk (no broadcast DMA).
  - o-proj bias enters via an appended ones row; the result DMAs straight
    from PSUM to HBM.
  - Phases are software-pipelined per 1536-column chunkset so the ACT
    engine (exp, the critical resource) never starves.
"""

import sys

sys.path.insert(0, "/opt/trn_rl_repo")

import numpy as np

LATENT = 64
COND = 768
HEADS = 8
DH = LATENT // HEADS  # 8
R = 8
SCALING = 1.0
SCALE = (LATENT / HEADS) ** -0.5
B = 16
HW = 4096
L = 77
N_CORES = 8
BPC = B // N_CORES  # 2 batches per core
CHUNK = 512
NCHUNK = HW // CHUNK  # 8
KCH = COND // 128  # 6 contraction chunks for k/v proj
CW = 78  # condT slot width (77 keys + 1 pad col)


def make_groups():
    """Bin-pack 8 heads x 77 keys into groups of <=128 score rows."""
    groups = []
    h, c = 0, 0
    while h < HEADS:
        segs = []
        off = 0
        while off < 128 and h < HEADS:
            take = min(L - c, 128 - off)
            segs.append((h, c, c + take, off))
            off += take
            c += take
            if c == L:
                h += 1
                c = 0
        groups.append(segs)
    return groups


GROUPS = make_groups()
NG = len(GROUPS)  # 5
GROUP_ROWS = [sum(c1 - c0 for _, c0, c1, _ in g) for g in GROUPS]
CSETS = [(0, 1536), (1536, 1536), (3072, 1024)]  # exp chunkset (off, width)

# f32r packed weights: col offsets
WOFF = {"wq2": 0}
WPACK_W = 128

# bf16 packed weights: col offsets
BOFF = {}
_o = 0
for _n, _w in [("vind", NG * 128), ("wkt", KCH * 64), ("wvt", KCH * 64),
               ("wot", 64), ("identbf", CW), ("onesL", CW), ("bv1", 64)]:
    BOFF[_n] = _o
    _o += _w
WBF_W = _o

# round-robin scatter DMA issue queues, per batch
KSCAT_Q = ["sync", "sync", "sync", "scalar", "scalar", "scalar",
           "gpsimd", "gpsimd"]


def _build_program(reps: int = 1):
    import concourse.mybir as mybir
    import concourse.tile as tile
    from concourse import bacc

    f32 = mybir.dt.float32
    f32r = mybir.dt.float32r
    bf16 = mybir.dt.bfloat16
    fp8 = mybir.dt.float8e4
    AF = mybir.ActivationFunctionType
    ALU = mybir.AluOpType
    DR = mybir.MatmulPerfMode.DoubleRow

    nc = bacc.Bacc("TRN2", target_bir_lowering=False, debug=False,
                   num_devices=N_CORES)

    def din(name, shape, dt=f32r):
        return nc.dram_tensor(name, shape, dt, kind="ExternalInput").ap()

    z_in = din("z", [BPC, LATENT, HW])
    cond_in = din("cond", [BPC, L, COND], f32)
    wpack = din("wpack", [128, WPACK_W])
    bias2 = din("bias2", [128, 3], f32)
    wbf = nc.dram_tensor("wbf", [128, WBF_W], bf16, kind="ExternalInput").ap()
    kz8 = nc.dram_tensor("kz8", [128, 2, NG * 128], fp8,
                         kind="ExternalInput").ap()
    out_d = nc.dram_tensor("out", [BPC, LATENT, HW], f32,
                           kind="ExternalOutput").ap()

    with tile.TileContext(nc) as tc:
        with (
            tc.tile_pool(name="persist", bufs=1) as pp,
            tc.tile_pool(name="work", bufs=2) as wp,
        ):
            # ---- persistent SBUF tensors ----
            x_sb = pp.tile([128, HW], f32r)          # z, both batches stacked
            qT8 = pp.tile([128, HW], fp8)
            kT8 = pp.tile([64, BPC * CW], fp8)
            kpack = pp.tile([128, 2, NG * 128], fp8)
            cond_bf = [pp.tile([L, COND], bf16, name=f"cond_bf{b}")
                       for b in range(BPC)]
            condT = pp.tile([128, BPC * KCH * CW], bf16)
            v_bf = [pp.tile([L, 64], bf16, name=f"v_bf{b}") for b in range(BPC)]
            vbig = [pp.tile([128, NG * 128], bf16, name=f"vbig{b}")
                    for b in range(BPC)]
            nrmd = pp.tile([64, HW], bf16)           # o-proj rhs
            osb = pp.tile([64, HW], f32)             # staged final output
            wp_sb = pp.tile([128, WPACK_W], f32r)
            wb_sb = pp.tile([128, WBF_W], bf16)
            bias_sb = pp.tile([128, 3], f32)

            # ---- static loads / initialization (once) ----
            # kpack zero-init is split per issue queue so each init precedes
            # exactly the scatters on its own (in-order) queue: heads 0-3 on
            # sync (cols 0:308), heads 4-7 + pad + s=1 subtile on scalar.
            nc.scalar.dma_start(out=wb_sb[:], in_=wbf[:])
            nc.sync.dma_start(out=wp_sb[:], in_=wpack[:])
            nc.sync.dma_start(out=bias_sb[:], in_=bias2[:])
            nc.sync.dma_start(out=kpack[:, 0, 0:L * 4], in_=kz8[:, 0, 0:L * 4])
            nc.scalar.dma_start(out=kpack[:, 0, L * 4:NG * 128],
                                in_=kz8[:, 0, L * 4:NG * 128])
            nc.scalar.dma_start(out=kpack[:, 1, :], in_=kz8[:, 1, :])

            def Wf(name, rows, width):
                o = WOFF[name]
                return wp_sb[0:rows, o:o + width]

            def Wb(name, rows, width):
                o = BOFF[name]
                return wb_sb[0:rows, o:o + width]

            wq_sb = Wf("wq2", 128, 128)
            bq_sb = bias_sb[:, 0:1]
            bk_sb = bias_sb[0:64, 1:2]
            bo_sb = bias_sb[0:64, 2:3]
            wkt_sb = Wb("wkt", 128, KCH * 64)
            wvt_sb = Wb("wvt", 128, KCH * 64)
            wot_sb = Wb("wot", 64, 64)
            ident_sb = Wb("identbf", L, CW)
            ones_sb = Wb("onesL", 1, CW)
            bv_sb = Wb("bv1", 1, 64)

            qdma = {"sync": nc.sync, "scalar": nc.scalar, "gpsimd": nc.gpsimd}

            for _rep in range(reps):
                # cond as casting gpsimd DMA (f32 HBM -> bf16 SBUF) issued
                # first on the Pool queue; vbig init follows on the same
                # queue as the v scatters (ordering). z cols 0:2048 go on
                # sync (urgent for early q-proj), 2048:4096 on scalar after
                # the k scatters (needed much later).
                for b in range(BPC):
                    nc.gpsimd.dma_start(out=cond_bf[b][:], in_=cond_in[b])
                for b in range(BPC):
                    nc.gpsimd.dma_start(
                        out=vbig[b][:],
                        in_=wbf[:, BOFF["vind"]:BOFF["vind"] + NG * 128])
                for q4 in range(2):
                    qs = slice(q4 * 1024, (q4 + 1) * 1024)
                    for b in range(BPC):
                        nc.sync.dma_start(out=x_sb[b * 64:(b + 1) * 64, qs],
                                          in_=z_in[b][:, qs])

                with tc.tile_pool(name="ps", bufs=1, space="PSUM") as ps0:
                    # ---- k projections (bf16) -> fp8 kpack, b0 first ----
                    for b in range(BPC):
                        tpb = ps0.tile([128, KCH * CW], bf16, tag="av", bufs=2,
                                       name=f"tp{b}")
                        for j in range(KCH):
                            nc.tensor.transpose(
                                out=tpb[:, j * CW:(j + 1) * CW],
                                in_=cond_bf[b][:, j * 128:(j + 1) * 128],
                                identity=ident_sb[:])
                        boff = b * KCH * CW
                        nc.vector.tensor_copy(
                            out=condT[:, boff:boff + KCH * CW], in_=tpb[:])
                        kps = ps0.tile([64, CW], f32, tag="av", bufs=2,
                                       name=f"kps{b}")
                        for j in range(KCH):
                            nc.tensor.matmul(
                                out=kps[:],
                                lhsT=wkt_sb[:, j * 64:(j + 1) * 64],
                                rhs=condT[:, boff + j * CW:boff + (j + 1) * CW],
                                start=(j == 0), stop=(j == KCH - 1))
                        nc.scalar.activation(
                            out=kT8[:, b * CW:(b + 1) * CW], in_=kps[:],
                            func=AF.Identity, bias=bk_sb[:])
                        for h in range(HEADS):
                            qdma["sync" if h < 4 else "scalar"].dma_start(
                                out=kpack[b * 64 + h * 8:b * 64 + h * 8 + 8,
                                          0, L * h:L * (h + 1)],
                                in_=kT8[h * 8:h * 8 + 8,
                                        b * CW:b * CW + L])

                    # remaining z columns (scalar queue, after k scatters)
                    for q4 in range(2, 4):
                        qs = slice(q4 * 1024, (q4 + 1) * 1024)
                        for b in range(BPC):
                            nc.scalar.dma_start(
                                out=x_sb[b * 64:(b + 1) * 64, qs],
                                in_=z_in[b][:, qs])

                    # ---- q projection + fp8 cast (+bias), 1024-wide ----
                    for c in range(4):
                        qps = ps0.tile([128, 1536], f32, tag="dots", bufs=2,
                                       name=f"qps{c}")
                        cs = slice(c * 1024, (c + 1) * 1024)
                        for i in range(2):
                            nc.tensor.matmul(
                                out=qps[:, i * CHUNK:(i + 1) * CHUNK],
                                lhsT=wq_sb[:],
                                rhs=x_sb[:, c * 1024 + i * CHUNK:
                                         c * 1024 + (i + 1) * CHUNK])
                        nc.vector.tensor_scalar(
                            out=qT8[:, cs], in0=qps[:, 0:1024],
                            scalar1=bq_sb[:], scalar2=None, op0=ALU.add)

                    # ---- v projections (key-major) + scatters (gpsimd) ----
                    for b in range(BPC):
                        boff = b * KCH * CW
                        vps = ps0.tile([L, 64], f32, tag="av", bufs=2,
                                       name=f"vps{b}")
                        for j in range(KCH):
                            nc.tensor.matmul(
                                out=vps[:],
                                lhsT=condT[:, boff + j * CW:boff + j * CW + L],
                                rhs=wvt_sb[:, j * 64:(j + 1) * 64],
                                start=(j == 0), stop=False)
                        nc.tensor.matmul(out=vps[:], lhsT=ones_sb[0:1, 0:L],
                                         rhs=bv_sb[:], start=False, stop=True)
                        nc.vector.tensor_copy(out=v_bf[b][:], in_=vps[:])
                        for g, segs in enumerate(GROUPS):
                            for (h, c0, c1, off) in segs:
                                nc.gpsimd.dma_start(
                                    out=vbig[b][off:off + (c1 - c0),
                                                g * 128 + h * 8:
                                                g * 128 + h * 8 + 8],
                                    in_=v_bf[b][c0:c1, h * 8:h * 8 + 8])

                    # ---- attention: 1-deep software pipeline per cset:
                    # issue scores+exp(k), then phase B of cset k-1, so the
                    # in-order PE queue never stalls on B's dependencies ----
                    def phase_b(b, doff, dw):
                        for c in range(dw // CHUNK):
                            cs = slice(doff + c * CHUNK,
                                       doff + (c + 1) * CHUNK)
                            avt = ps0.tile([128, CHUNK], f32, tag="av",
                                           bufs=2, name=f"av_{b}_{c}")
                            for g in range(NG):
                                rg = GROUP_ROWS[g]
                                nc.tensor.matmul(
                                    out=avt[:],
                                    lhsT=vbig[b][0:rg, g * 128:(g + 1) * 128],
                                    rhs=exp_tiles[b][g][0:rg, cs],
                                    start=(g == 0), stop=(g == NG - 1))
                            rcp = wp.tile([64, CHUNK], f32, tag="rcp",
                                          name=f"rcp_{b}_{c}")
                            nc.vector.reciprocal(out=rcp[:],
                                                 in_=avt[64:128, :])
                            nc.vector.tensor_mul(out=nrmd[:, cs],
                                                 in0=avt[0:64, :], in1=rcp[:])
                            opt = ps0.tile([64, CHUNK], f32, tag="av",
                                           bufs=2, name=f"op_{b}_{c}")
                            nc.tensor.matmul(out=opt[:], lhsT=wot_sb[:],
                                             rhs=nrmd[:, cs])
                            nc.vector.tensor_scalar(
                                out=osb[:, cs], in0=opt[:], scalar1=bo_sb[:],
                                scalar2=None, op0=ALU.add)
                            qdma["sync" if c % 2 == 0 else "scalar"].dma_start(
                                out=out_d[b][:, cs], in_=osb[:, cs])

                    exp_tiles = {}
                    prev = None
                    for b in range(BPC):
                        p = slice(b * 64, (b + 1) * 64)
                        exp_tiles[b] = [wp.tile([128, HW], bf16, tag=f"exp{g}",
                                                bufs=2, name=f"exp_{b}_{g}")
                                        for g in range(NG)]
                        for (doff, dw) in CSETS:
                            for g in range(NG):
                                rg = GROUP_ROWS[g]
                                dps = ps0.tile([128, 1536], f32, tag="dots",
                                               bufs=2, name=f"dps_{b}_{g}")
                                for i in range(dw // CHUNK):
                                    pos = doff + i * CHUNK
                                    nc.tensor.matmul(
                                        out=dps[0:rg,
                                                i * CHUNK:(i + 1) * CHUNK],
                                        lhsT=kpack[p, :, g * 128:g * 128 + rg],
                                        rhs=qT8[p, pos:pos + CHUNK][:, None, :]
                                            .broadcast_to([64, 2, CHUNK]),
                                        perf_mode=DR)
                                nc.scalar.activation(
                                    out=exp_tiles[b][g][0:rg, doff:doff + dw],
                                    in_=dps[0:rg, 0:dw], func=AF.Exp)
                            if prev is not None:
                                phase_b(*prev)
                            prev = (b, doff, dw)
                    phase_b(*prev)
    nc.compile()
    return nc


def _prep_weights(Wq, bq, Aq, Bq, Wk, bk, Ak, Bk, Wv, bv, Av, Bv,
                  Wo, bo, Ao, Bo):
    import ml_dtypes

    def eff(W, A, Bm):
        return (W + SCALING * (Bm @ A)).astype(np.float32)

    Wq_s = eff(Wq, Aq, Bq) * SCALE
    bq_s = (bq * SCALE).astype(np.float32)
    Wk_e, Wv_e, Wo_e = eff(Wk, Ak, Bk), eff(Wv, Av, Bv), eff(Wo, Ao, Bo)

    def chunked_T(We):  # [64, 768] -> [128, 6*64]
        WT = We.T.reshape(KCH, 128, 64)
        return np.ascontiguousarray(
            WT.transpose(1, 0, 2).reshape(128, KCH * 64)).astype(np.float32)

    def _blockdiag(WT):
        out = np.zeros((128, 128), np.float32)
        out[0:64, 0:64] = WT
        out[64:128, 64:128] = WT
        return out

    wpk = np.zeros((128, WPACK_W), np.float32)
    wpk[:, WOFF["wq2"]:WOFF["wq2"] + 128] = _blockdiag(Wq_s.T)
    bias2 = np.zeros((128, 3), np.float32)
    bias2[:, 0] = np.concatenate([bq_s, bq_s])
    bias2[0:64, 1] = bk.astype(np.float32)
    bias2[0:64, 2] = bo.astype(np.float32)

    wbf = np.zeros((128, WBF_W), np.float32)

    def putb(name, arr):
        r, w = arr.shape
        wbf[0:r, BOFF[name]:BOFF[name] + w] = arr

    vind = np.zeros((128, NG * 128), np.float32)
    for g, segs in enumerate(GROUPS):
        for (h, c0, c1, off) in segs:
            vind[off:off + (c1 - c0),
                 g * 128 + 64 + h * 8:g * 128 + 64 + h * 8 + 8] = 1.0
    putb("vind", vind)
    putb("wkt", chunked_T(Wk_e))
    putb("wvt", chunked_T(Wv_e))
    putb("wot", Wo_e.T.astype(np.float32))
    putb("identbf", np.concatenate([np.eye(L, dtype=np.float32),
                                    np.zeros((L, CW - L), np.float32)], 1))
    putb("onesL", np.ones((1, CW), np.float32))
    putb("bv1", bv[None, :].astype(np.float32))
    return {"wpack": wpk, "bias2": bias2, "wbf": wbf.astype(ml_dtypes.bfloat16),
            "kz8": np.zeros((128, 2, NG * 128), ml_dtypes.float8_e4m3)}


class _Runner:
    """Builds the sharded jit once; supports repeated timed executions."""

    def __init__(self, nc, n_cores):
        import jax
        import concourse.mybir as mybir
        from jax.sharding import Mesh, PartitionSpec
        from jax.experimental.shard_map import shard_map
        from concourse import bass2jax
        from concourse.bass2jax import _bass_exec_p, install_neuronx_cc_hook

        install_neuronx_cc_hook()
        self.jax = jax
        self.nc = nc
        self.n = n_cores
        pname = nc.partition_id_tensor.name if nc.partition_id_tensor else None
        in_names, out_names, out_avals, zeros = [], [], [], []
        for alloc in nc.m.functions[0].allocations:
            if not isinstance(alloc, mybir.MemoryLocationSet):
                continue
            name = alloc.memorylocations[0].name
            if alloc.kind == "ExternalInput":
                if name != pname:
                    in_names.append(name)
            elif alloc.kind == "ExternalOutput":
                out_names.append(name)
                shape = tuple(alloc.tensor_shape)
                dt = mybir.dt.np(alloc.dtype)
                out_avals.append(jax.core.ShapedArray(shape, dt))
                zeros.append(np.zeros(shape, dt))
        self.in_names, self.out_names = in_names, out_names
        self.out_avals, self.zeros = out_avals, zeros
        all_in = in_names + out_names + ([pname] if pname else [])

        def _body(*args):
            ops = list(args)
            if pname:
                ops.append(bass2jax.partition_id_tensor())
            return tuple(_bass_exec_p.bind(
                *ops, out_avals=tuple(out_avals), in_names=tuple(all_in),
                out_names=tuple(out_names), lowering_input_output_aliases=(),
                sim_require_finite=True, sim_require_nnan=True, nc=nc))

        devices = jax.devices()[:n_cores]
        mesh = Mesh(np.asarray(devices), ("core",))
        nin = len(in_names) + len(zeros)
        self.fn = jax.jit(
            shard_map(_body, mesh=mesh, in_specs=(PartitionSpec("core"),) * nin,
                      out_specs=(PartitionSpec("core"),) * len(out_names),
                      check_rep=False),
            keep_unused=True)
        self._dev = None

    def set_inputs(self, in_maps):
        jax, n = self.jax, self.n
        cat = [np.concatenate([np.asarray(in_maps[c][nm]) for c in range(n)], 0)
               for nm in self.in_names]
        catz = [np.zeros((n * z.shape[0], *z.shape[1:]), z.dtype)
                for z in self.zeros]
        self._dev = [jax.device_put(a) for a in cat + catz]

    def run(self):
        out = self.fn(*self._dev)
        self.jax.block_until_ready(out)
        return out

    def results(self, out):
        n = self.n
        return [{nm: np.asarray(out[i]).reshape(n, *self.out_avals[i].shape)[c]
                 for i, nm in enumerate(self.out_names)}
                for c in range(n)]


_STATE = {}


def _get_runner(reps: int = 1):
    key = ("runner", reps)
    if key not in _STATE:
        nc = _build_program(reps)
        _STATE[key] = _Runner(nc, N_CORES)
    return _STATE[key]


def kernel(z, cond, Wq, bq, Aq, Bq, Wk, bk, Ak, Bk, Wv, bv, Av, Bv,
           Wo, bo, Ao, Bo):
    z = np.asarray(z, np.float32)
    cond = np.asarray(cond, np.float32)
    w = _prep_weights(np.asarray(Wq), np.asarray(bq), np.asarray(Aq),
                      np.asarray(Bq), np.asarray(Wk), np.asarray(bk),
                      np.asarray(Ak), np.asarray(Bk), np.asarray(Wv),
                      np.asarray(bv), np.asarray(Av), np.asarray(Bv),
                      np.asarray(Wo), np.asarray(bo), np.asarray(Ao),
                      np.asarray(Bo))
    r = _get_runner()
    in_maps = []
    for c in range(N_CORES):
        m = dict(w)
        m["z"] = np.ascontiguousarray(
            z[c * BPC:(c + 1) * BPC].reshape(BPC, LATENT, HW))
        m["cond"] = np.ascontiguousarray(cond[c * BPC:(c + 1) * BPC])
        in_maps.append(m)
    r.set_inputs(in_maps)
    res = r.results(r.run())
    out = np.empty((B, LATENT, 64, 64), np.float32)
    for c in range(N_CORES):
        out[c * BPC:(c + 1) * BPC] = res[c]["out"].reshape(BPC, LATENT, 64, 64)
    return out


# revision 42
# speedup vs baseline: 1.0936x; 1.0936x over previous
"""Trainium2 Bass kernel for nn_CrossAttention (LoRA cross-attention).

Sharding: data-parallel over batch — 16 batches across 8 cores, 2 per core.

Per-core strategy (all feature-major, no data transposes of activations):
  - z arrives [C=64, HW=4096] per batch (native layout) = q-proj rhs.
  - q is projected f32r then cast (+bias) to fp8e4 on DVE; k is projected
    bf16 and cast (+bias) to fp8e4 on ACT. Scores then run as fp8
    DoubleRow matmuls (0.5 cyc/col) against a block-diagonal packed K
    stationary: all 8 heads x 77 keys bin-packed into 5 groups of <=128
    PSUM partitions, so exp() runs at full 128-lane ACT utilization.
  - The augmented block-diagonal V stationary (bf16) carries 64 replicated
    indicator columns, so the AV PSUM tile holds features in rows 0:64 and
    the per-head softmax sums replicated x8 in rows 64:128 — normalization
    is then a single DVE divide per chunk (no broadcast DMA).
  - o-proj bias enters via an appended ones row; the result DMAs straight
    from PSUM to HBM.
  - Phases are software-pipelined per 1536-column chunkset so the ACT
    engine (exp, the critical resource) never starves.
"""

import sys

sys.path.insert(0, "/opt/trn_rl_repo")

import numpy as np

LATENT = 64
COND = 768
HEADS = 8
DH = LATENT // HEADS  # 8
R = 8
SCALING = 1.0
SCALE = (LATENT / HEADS) ** -0.5
B = 16
HW = 4096
L = 77
N_CORES = 8
BPC = B // N_CORES  # 2 batches per core
CHUNK = 512
NCHUNK = HW // CHUNK  # 8
KCH = COND // 128  # 6 contraction chunks for k/v proj
CW = 78  # condT slot width (77 keys + 1 pad col)


def make_groups():
    """Bin-pack 8 heads x 77 keys into groups of <=128 score rows."""
    groups = []
    h, c = 0, 0
    while h < HEADS:
        segs = []
        off = 0
        while off < 128 and h < HEADS:
            take = min(L - c, 128 - off)
            segs.append((h, c, c + take, off))
            off += take
            c += take
            if c == L:
                h += 1
                c = 0
        groups.append(segs)
    return groups


GROUPS = make_groups()
NG = len(GROUPS)  # 5
GROUP_ROWS = [sum(c1 - c0 for _, c0, c1, _ in g) for g in GROUPS]
CSETS = [(0, 1536), (1536, 1536), (3072, 1024)]  # exp chunkset (off, width)

# f32r packed weights: col offsets
WOFF = {"wq2": 0}
WPACK_W = 128

# bf16 packed weights: col offsets
BOFF = {}
_o = 0
for _n, _w in [("vind", NG * 128), ("wkt", KCH * 64), ("wvt", KCH * 64),
               ("wot", 64), ("identbf", CW), ("onesL", CW), ("bv1", 64)]:
    BOFF[_n] = _o
    _o += _w
WBF_W = _o

# round-robin scatter DMA issue queues, per batch
KSCAT_Q = ["sync", "sync", "sync", "scalar", "scalar", "scalar",
           "gpsimd", "gpsimd"]


def _build_program(reps: int = 1):
    import concourse.mybir as mybir
    import concourse.tile as tile
    from concourse import bacc

    f32 = mybir.dt.float32
    f32r = mybir.dt.float32r
    bf16 = mybir.dt.bfloat16
    fp8 = mybir.dt.float8e4
    AF = mybir.ActivationFunctionType
    ALU = mybir.AluOpType
    DR = mybir.MatmulPerfMode.DoubleRow

    nc = bacc.Bacc("TRN2", target_bir_lowering=False, debug=False,
                   num_devices=N_CORES)

    def din(name, shape, dt=f32r):
        return nc.dram_tensor(name, shape, dt, kind="ExternalInput").ap()

    z_in = din("z", [BPC, LATENT, HW])
    cond_in = din("cond", [BPC, L, COND], f32)
    wpack = din("wpack", [128, WPACK_W])
    bias2 = din("bias2", [128, 3], f32)
    wbf = nc.dram_tensor("wbf", [128, WBF_W], bf16, kind="ExternalInput").ap()
    kz8 = nc.dram_tensor("kz8", [128, 2, NG * 128], fp8,
                         kind="ExternalInput").ap()
    out_d = nc.dram_tensor("out", [BPC, LATENT, HW], f32,
                           kind="ExternalOutput").ap()

    with tile.TileContext(nc) as tc:
        with (
            tc.tile_pool(name="persist", bufs=1) as pp,
            tc.tile_pool(name="work", bufs=2) as wp,
        ):
            # ---- persistent SBUF tensors ----
            x_sb = pp.tile([128, HW], f32r)          # z, both batches stacked
            qT8 = pp.tile([128, HW], fp8)
            kT8 = pp.tile([64, BPC * CW], fp8)
            kpack = pp.tile([128, 2, NG * 128], fp8)
            cond_bf = [pp.tile([L, COND], bf16, name=f"cond_bf{b}")
                       for b in range(BPC)]
            condT = pp.tile([128, BPC * KCH * CW], bf16)
            v_bf = [pp.tile([L, 64], bf16, name=f"v_bf{b}") for b in range(BPC)]
            vbig = [pp.tile([128, NG * 128], bf16, name=f"vbig{b}")
                    for b in range(BPC)]
            nrmd = pp.tile([64, HW], bf16)           # o-proj rhs
            osb = pp.tile([64, HW], f32)             # staged final output
            wp_sb = pp.tile([128, WPACK_W], f32r)
            wb_sb = pp.tile([128, WBF_W], bf16)
            bias_sb = pp.tile([128, 3], f32)

            # ---- static loads / initialization (once) ----
            # kpack zero-init is split per issue queue so each init precedes
            # exactly the scatters on its own (in-order) queue: heads 0-3 on
            # sync (cols 0:308), heads 4-7 + pad + s=1 subtile on scalar.
            nc.scalar.dma_start(out=wb_sb[:], in_=wbf[:])
            nc.sync.dma_start(out=wp_sb[:], in_=wpack[:])
            nc.sync.dma_start(out=bias_sb[:], in_=bias2[:])
            nc.sync.dma_start(out=kpack[:, 0, 0:L * 4], in_=kz8[:, 0, 0:L * 4])
            nc.scalar.dma_start(out=kpack[:, 0, L * 4:NG * 128],
                                in_=kz8[:, 0, L * 4:NG * 128])
            nc.scalar.dma_start(out=kpack[:, 1, :], in_=kz8[:, 1, :])

            def Wf(name, rows, width):
                o = WOFF[name]
                return wp_sb[0:rows, o:o + width]

            def Wb(name, rows, width):
                o = BOFF[name]
                return wb_sb[0:rows, o:o + width]

            wq_sb = Wf("wq2", 128, 128)
            bq_sb = bias_sb[:, 0:1]
            bk_sb = bias_sb[0:64, 1:2]
            bo_sb = bias_sb[0:64, 2:3]
            wkt_sb = Wb("wkt", 128, KCH * 64)
            wvt_sb = Wb("wvt", 128, KCH * 64)
            wot_sb = Wb("wot", 64, 64)
            ident_sb = Wb("identbf", L, CW)
            ones_sb = Wb("onesL", 1, CW)
            bv_sb = Wb("bv1", 1, 64)

            qdma = {"sync": nc.sync, "scalar": nc.scalar, "gpsimd": nc.gpsimd}

            for _rep in range(reps):
                # cond as casting gpsimd DMA (f32 HBM -> bf16 SBUF) issued
                # first on the Pool queue; vbig init follows on the same
                # queue as the v scatters (ordering). z cols 0:2048 go on
                # sync (urgent for early q-proj), 2048:4096 on scalar after
                # the k scatters (needed much later).
                for b in range(BPC):
                    nc.gpsimd.dma_start(out=cond_bf[b][:], in_=cond_in[b])
                for b in range(BPC):
                    nc.gpsimd.dma_start(
                        out=vbig[b][:],
                        in_=wbf[:, BOFF["vind"]:BOFF["vind"] + NG * 128])
                for q4 in range(2):
                    qs = slice(q4 * 1024, (q4 + 1) * 1024)
                    for b in range(BPC):
                        nc.sync.dma_start(out=x_sb[b * 64:(b + 1) * 64, qs],
                                          in_=z_in[b][:, qs])

                with tc.tile_pool(name="ps", bufs=1, space="PSUM") as ps0:

                    def k_chain(b):
                        # transpose cond -> condT, k-proj, fp8 cast, scatter
                        tpb = ps0.tile([128, KCH * CW], bf16, tag="av",
                                       bufs=2, name=f"tp{b}")
                        for j in range(KCH):
                            nc.tensor.transpose(
                                out=tpb[:, j * CW:(j + 1) * CW],
                                in_=cond_bf[b][:, j * 128:(j + 1) * 128],
                                identity=ident_sb[:])
                        boff = b * KCH * CW
                        nc.vector.tensor_copy(
                            out=condT[:, boff:boff + KCH * CW], in_=tpb[:])
                        kps = ps0.tile([64, CW], f32, tag="av", bufs=2,
                                       name=f"kps{b}")
                        for j in range(KCH):
                            nc.tensor.matmul(
                                out=kps[:],
                                lhsT=wkt_sb[:, j * 64:(j + 1) * 64],
                                rhs=condT[:, boff + j * CW:boff + (j + 1) * CW],
                                start=(j == 0), stop=(j == KCH - 1))
                        nc.scalar.activation(
                            out=kT8[:, b * CW:(b + 1) * CW], in_=kps[:],
                            func=AF.Identity, bias=bk_sb[:])
                        for h in range(HEADS):
                            qdma["sync" if h < 4 else "scalar"].dma_start(
                                out=kpack[b * 64 + h * 8:b * 64 + h * 8 + 8,
                                          0, L * h:L * (h + 1)],
                                in_=kT8[h * 8:h * 8 + 8, b * CW:b * CW + L])

                    def v_chain(b):
                        boff = b * KCH * CW
                        vps = ps0.tile([L, 64], f32, tag="av", bufs=2,
                                       name=f"vps{b}")
                        for j in range(KCH):
                            nc.tensor.matmul(
                                out=vps[:],
                                lhsT=condT[:, boff + j * CW:boff + j * CW + L],
                                rhs=wvt_sb[:, j * 64:(j + 1) * 64],
                                start=(j == 0), stop=False)
                        nc.tensor.matmul(out=vps[:], lhsT=ones_sb[0:1, 0:L],
                                         rhs=bv_sb[:], start=False, stop=True)
                        nc.vector.tensor_copy(out=v_bf[b][:], in_=vps[:])
                        for g, segs in enumerate(GROUPS):
                            for (h, c0, c1, off) in segs:
                                nc.gpsimd.dma_start(
                                    out=vbig[b][off:off + (c1 - c0),
                                                g * 128 + h * 8:
                                                g * 128 + h * 8 + 8],
                                    in_=v_bf[b][c0:c1, h * 8:h * 8 + 8])

                    def qproj(c):
                        qps = ps0.tile([128, 1536], f32, tag="dots", bufs=2,
                                       name=f"qps{c}")
                        cs = slice(c * 1024, (c + 1) * 1024)
                        for i in range(2):
                            nc.tensor.matmul(
                                out=qps[:, i * CHUNK:(i + 1) * CHUNK],
                                lhsT=wq_sb[:],
                                rhs=x_sb[:, c * 1024 + i * CHUNK:
                                         c * 1024 + (i + 1) * CHUNK])
                        nc.vector.tensor_scalar(
                            out=qT8[:, cs], in0=qps[:, 0:1024],
                            scalar1=bq_sb[:], scalar2=None, op0=ALU.add)

                    exp_tiles = {}

                    def scores_exp(b, doff, dw):
                        p = slice(b * 64, (b + 1) * 64)
                        for g in range(NG):
                            rg = GROUP_ROWS[g]
                            dps = ps0.tile([128, 1536], f32, tag="dots",
                                           bufs=2, name=f"dps_{b}_{g}")
                            for i in range(dw // CHUNK):
                                pos = doff + i * CHUNK
                                nc.tensor.matmul(
                                    out=dps[0:rg, i * CHUNK:(i + 1) * CHUNK],
                                    lhsT=kpack[p, :, g * 128:g * 128 + rg],
                                    rhs=qT8[p, pos:pos + CHUNK][:, None, :]
                                        .broadcast_to([64, 2, CHUNK]),
                                    perf_mode=DR)
                            nc.scalar.activation(
                                out=exp_tiles[b][g][0:rg, doff:doff + dw],
                                in_=dps[0:rg, 0:dw], func=AF.Exp)

                    def phase_b(b, doff, dw):
                        for c in range(dw // CHUNK):
                            cs = slice(doff + c * CHUNK, doff + (c + 1) * CHUNK)
                            avt = ps0.tile([128, CHUNK], f32, tag="av",
                                           bufs=2, name=f"av_{b}_{c}")
                            for g in range(NG):
                                rg = GROUP_ROWS[g]
                                nc.tensor.matmul(
                                    out=avt[:],
                                    lhsT=vbig[b][0:rg, g * 128:(g + 1) * 128],
                                    rhs=exp_tiles[b][g][0:rg, cs],
                                    start=(g == 0), stop=(g == NG - 1))
                            rcp = wp.tile([64, CHUNK], f32, tag="rcp",
                                          name=f"rcp_{b}_{c}")
                            nc.vector.reciprocal(out=rcp[:], in_=avt[64:128, :])
                            nc.vector.tensor_mul(out=nrmd[:, cs],
                                                 in0=avt[0:64, :], in1=rcp[:])
                            opt = ps0.tile([64, CHUNK], f32, tag="av",
                                           bufs=2, name=f"op_{b}_{c}")
                            nc.tensor.matmul(out=opt[:], lhsT=wot_sb[:],
                                             rhs=nrmd[:, cs])
                            nc.vector.tensor_scalar(
                                out=osb[:, cs], in0=opt[:], scalar1=bo_sb[:],
                                scalar2=None, op0=ALU.add)
                            qdma["sync" if c % 2 == 0 else "scalar"].dma_start(
                                out=out_d[b][:, cs], in_=osb[:, cs])

                    for b in range(BPC):
                        exp_tiles[b] = [wp.tile([128, HW], bf16, tag=f"exp{g}",
                                                bufs=2, name=f"exp_{b}_{g}")
                                        for g in range(NG)]

                    # hand-scheduled emission: b0's k chain and first two q
                    # chunks come first, b1's projections ride between b0's
                    # chunksets, phase B trails scores/exp by one chunkset.
                    k_chain(0)
                    qproj(0)
                    qproj(1)
                    v_chain(0)
                    for q4 in range(2, 4):
                        qs = slice(q4 * 1024, (q4 + 1) * 1024)
                        for b in range(BPC):
                            nc.sync.dma_start(
                                out=x_sb[b * 64:(b + 1) * 64, qs],
                                in_=z_in[b][:, qs])
                    scores_exp(0, *CSETS[0])
                    qproj(2)
                    qproj(3)
                    k_chain(1)
                    v_chain(1)
                    work = [(b, cs) for b in range(BPC) for cs in CSETS]
                    for i in range(1, len(work)):
                        b, cs = work[i]
                        scores_exp(b, *cs)
                        pb, pcs = work[i - 1]
                        phase_b(pb, *pcs)
                    lb, lcs = work[-1]
                    phase_b(lb, *lcs)
    nc.compile()
    return nc


def _prep_weights(Wq, bq, Aq, Bq, Wk, bk, Ak, Bk, Wv, bv, Av, Bv,
                  Wo, bo, Ao, Bo):
    import ml_dtypes

    def eff(W, A, Bm):
        return (W + SCALING * (Bm @ A)).astype(np.float32)

    Wq_s = eff(Wq, Aq, Bq) * SCALE
    bq_s = (bq * SCALE).astype(np.float32)
    Wk_e, Wv_e, Wo_e = eff(Wk, Ak, Bk), eff(Wv, Av, Bv), eff(Wo, Ao, Bo)

    def chunked_T(We):  # [64, 768] -> [128, 6*64]
        WT = We.T.reshape(KCH, 128, 64)
        return np.ascontiguousarray(
            WT.transpose(1, 0, 2).reshape(128, KCH * 64)).astype(np.float32)

    def _blockdiag(WT):
        out = np.zeros((128, 128), np.float32)
        out[0:64, 0:64] = WT
        out[64:128, 64:128] = WT
        return out

    wpk = np.zeros((128, WPACK_W), np.float32)
    wpk[:, WOFF["wq2"]:WOFF["wq2"] + 128] = _blockdiag(Wq_s.T)
    bias2 = np.zeros((128, 3), np.float32)
    bias2[:, 0] = np.concatenate([bq_s, bq_s])
    bias2[0:64, 1] = bk.astype(np.float32)
    bias2[0:64, 2] = bo.astype(np.float32)

    wbf = np.zeros((128, WBF_W), np.float32)

    def putb(name, arr):
        r, w = arr.shape
        wbf[0:r, BOFF[name]:BOFF[name] + w] = arr

    vind = np.zeros((128, NG * 128), np.float32)
    for g, segs in enumerate(GROUPS):
        for (h, c0, c1, off) in segs:
            vind[off:off + (c1 - c0),
                 g * 128 + 64 + h * 8:g * 128 + 64 + h * 8 + 8] = 1.0
    putb("vind", vind)
    putb("wkt", chunked_T(Wk_e))
    putb("wvt", chunked_T(Wv_e))
    putb("wot", Wo_e.T.astype(np.float32))
    putb("identbf", np.concatenate([np.eye(L, dtype=np.float32),
                                    np.zeros((L, CW - L), np.float32)], 1))
    putb("onesL", np.ones((1, CW), np.float32))
    putb("bv1", bv[None, :].astype(np.float32))
    return {"wpack": wpk, "bias2": bias2, "wbf": wbf.astype(ml_dtypes.bfloat16),
            "kz8": np.zeros((128, 2, NG * 128), ml_dtypes.float8_e4m3)}


class _Runner:
    """Builds the sharded jit once; supports repeated timed executions."""

    def __init__(self, nc, n_cores):
        import jax
        import concourse.mybir as mybir
        from jax.sharding import Mesh, PartitionSpec
        from jax.experimental.shard_map import shard_map
        from concourse import bass2jax
        from concourse.bass2jax import _bass_exec_p, install_neuronx_cc_hook

        install_neuronx_cc_hook()
        self.jax = jax
        self.nc = nc
        self.n = n_cores
        pname = nc.partition_id_tensor.name if nc.partition_id_tensor else None
        in_names, out_names, out_avals, zeros = [], [], [], []
        for alloc in nc.m.functions[0].allocations:
            if not isinstance(alloc, mybir.MemoryLocationSet):
                continue
            name = alloc.memorylocations[0].name
            if alloc.kind == "ExternalInput":
                if name != pname:
                    in_names.append(name)
            elif alloc.kind == "ExternalOutput":
                out_names.append(name)
                shape = tuple(alloc.tensor_shape)
                dt = mybir.dt.np(alloc.dtype)
                out_avals.append(jax.core.ShapedArray(shape, dt))
                zeros.append(np.zeros(shape, dt))
        self.in_names, self.out_names = in_names, out_names
        self.out_avals, self.zeros = out_avals, zeros
        all_in = in_names + out_names + ([pname] if pname else [])

        def _body(*args):
            ops = list(args)
            if pname:
                ops.append(bass2jax.partition_id_tensor())
            return tuple(_bass_exec_p.bind(
                *ops, out_avals=tuple(out_avals), in_names=tuple(all_in),
                out_names=tuple(out_names), lowering_input_output_aliases=(),
                sim_require_finite=True, sim_require_nnan=True, nc=nc))

        devices = jax.devices()[:n_cores]
        mesh = Mesh(np.asarray(devices), ("core",))
        nin = len(in_names) + len(zeros)
        self.fn = jax.jit(
            shard_map(_body, mesh=mesh, in_specs=(PartitionSpec("core"),) * nin,
                      out_specs=(PartitionSpec("core"),) * len(out_names),
                      check_rep=False),
            keep_unused=True)
        self._dev = None

    def set_inputs(self, in_maps):
        jax, n = self.jax, self.n
        cat = [np.concatenate([np.asarray(in_maps[c][nm]) for c in range(n)], 0)
               for nm in self.in_names]
        catz = [np.zeros((n * z.shape[0], *z.shape[1:]), z.dtype)
                for z in self.zeros]
        self._dev = [jax.device_put(a) for a in cat + catz]

    def run(self):
        out = self.fn(*self._dev)
        self.jax.block_until_ready(out)
        return out

    def results(self, out):
        n = self.n
        return [{nm: np.asarray(out[i]).reshape(n, *self.out_avals[i].shape)[c]
                 for i, nm in enumerate(self.out_names)}
                for c in range(n)]


_STATE = {}


def _get_runner(reps: int = 1):
    key = ("runner", reps)
    if key not in _STATE:
        nc = _build_program(reps)
        _STATE[key] = _Runner(nc, N_CORES)
    return _STATE[key]


def kernel(z, cond, Wq, bq, Aq, Bq, Wk, bk, Ak, Bk, Wv, bv, Av, Bv,
           Wo, bo, Ao, Bo):
    z = np.asarray(z, np.float32)
    cond = np.asarray(cond, np.float32)
    w = _prep_weights(np.asarray(Wq), np.asarray(bq), np.asarray(Aq),
                      np.asarray(Bq), np.asarray(Wk), np.asarray(bk),
                      np.asarray(Ak), np.asarray(Bk), np.asarray(Wv),
                      np.asarray(bv), np.asarray(Av), np.asarray(Bv),
                      np.asarray(Wo), np.asarray(bo), np.asarray(Ao),
                      np.asarray(Bo))
    r = _get_runner()
    in_maps = []
    for c in range(N_CORES):
        m = dict(w)
        m["z"] = np.ascontiguousarray(
            z[c * BPC:(c + 1) * BPC].reshape(BPC, LATENT, HW))
        m["cond"] = np.ascontiguousarray(cond[c * BPC:(c + 1) * BPC])
        in_maps.append(m)
    r.set_inputs(in_maps)
    res = r.results(r.run())
    out = np.empty((B, LATENT, 64, 64), np.float32)
    for c in range(N_CORES):
        out[c * BPC:(c + 1) * BPC] = res[c]["out"].reshape(BPC, LATENT, 64, 64)
    return out
